# revision 24
# baseline (speedup 1.0000x reference)
"""GIN (3-layer) Trainium2 Bass kernel, 8-core SPMD — v2.

Sharding: nodes (and their incident edges, by dst) are partitioned across the
8 cores; segment_sum is computed locally per dst shard; node features are
exchanged between layers with chunked AllGathers; MLP weights are replicated.

v2 changes over the baseline:
  - gathered node features travel in fp8 (e4m3): halves the dominant HBM
    gather traffic and the inter-core AllGather bytes (rel-err budget checked
    against the reference: ~6e-3 vs 2e-2 tolerance).
  - one-hot segment-sum matmuls run in fp8 with DoubleRow perf mode.
  - selector matrices S are fp8 and stay resident in SBUF across all layers.
  - gathers use prepare_only descriptors + trigger_dma so descriptor
    generation runs ahead of the data dependency (which moves to the trigger).
  - the h AllGather is split into 3 parts aligned with the MLP node chunks;
    each part is a separate Shared tile, and the next layer's gathers are
    split per part so part-p gathers fire as soon as AG part p lands.
"""

import os
import sys
from contextlib import ExitStack

import numpy as np

for _p in ("/opt/trn_rl_repo", "/root/.axon_site/_ro/trn_rl_repo"):
    if os.path.isdir(_p) and _p not in sys.path:
        sys.path.append(_p)

import ml_dtypes

N_NODES = 10000
N_EDGES = 160000
D = 512
N_LAYERS = 3
CORES = 8
SHARD = N_NODES // CORES          # 1250 nodes per core
PADS = 1280                       # padded shard (multiple of 128)
PADN = CORES * PADS               # padded full node count (10240)
NB = PADS // 128                  # dst blocks per core (10)

# AllGather parts: (local row lo, local row hi) — fired at MLP chunk bounds.
PARTS = [(0, 512), (512, 1280)]
NCHUNK = [(0, 512), (512, 512), (1024, 256)]  # node-dim tiles for MLP / AG
MAXCH = 8                         # max chunks per gather (SWDGE ring: 1024 descs)
LOOKAHEAD = 2                     # prep emission distance (units) behind triggers

BF16 = ml_dtypes.bfloat16
FP8 = ml_dtypes.float8_e4m3

# Results of the last kernel() call (BassKernelResults) for the test harness.
LAST_RESULTS = None


def _prep_host(x, edge_index, Ws, bs):
    """Per-core input maps + per-(block, part) chunk counts (uniform across
    cores). Gather rows are part-local: row = rank * part_rows + local offset.
    """
    x = np.asarray(x, np.float32)
    src = np.asarray(edge_index[0], np.int64)
    dst = np.asarray(edge_index[1], np.int64)
    Ws = np.asarray(Ws, np.float32)
    bs = np.asarray(bs, np.float32)

    r_src = src // SHARD
    i_src = src % SHARD
    pedges = np.array([lo for lo, _ in PARTS[1:]], np.int64)
    part = np.searchsorted(pedges, i_src, side="right")
    pw = np.array([hi - lo for lo, hi in PARTS], np.int64)
    plo = np.array([lo for lo, _ in PARTS], np.int64)
    row_local = r_src * pw[part] + (i_src - plo[part])
    ROWMOD = 8192  # > max row_local; keeps dedup keys collision-free

    owner = dst // SHARD
    li = dst % SHARD
    blk = li // 128
    slot = li - blk * 128

    # Per (core, block, part) unique-src counts -> chunk counts (max over
    # cores so the SPMD program is uniform).
    NPART = len(PARTS)
    key = ((owner * NB + blk) * NPART + part) * ROWMOD + row_local
    ucnt = np.zeros(CORES * NB * NPART, np.int64)
    kb = np.unique(key) // ROWMOD
    np.add.at(ucnt, kb, 1)
    ucnt = ucnt.reshape(CORES, NB, NPART)
    cnt = np.maximum(1, -(-ucnt.max(axis=0) // 128))   # [NB, NPART] chunks
    off = np.zeros((NB, NPART + 1), np.int64)
    off[:, 1:] = np.cumsum(cnt, axis=1)
    CPAD = off[:, -1]                                  # padded chunks per block
    CPADMAX = int(CPAD.max())

    # Full part-layout x in fp8 (gather source for layer 0), shared by cores.
    xg_pad = np.zeros((PADN, D), FP8)
    base = 0
    part_base = []
    for (lo, hi) in PARTS:
        part_base.append(base)
        for r in range(CORES):
            hi_c = min(hi, SHARD)
            xg_pad[base + r * (hi - lo): base + r * (hi - lo) + (hi_c - lo)] = \
                x[r * SHARD + lo: r * SHARD + hi_c].astype(FP8)
        base += CORES * (hi - lo)

    Wd = np.ascontiguousarray(Ws.reshape(2 * N_LAYERS, D, D).astype(BF16))
    bT = np.ascontiguousarray(
        bs.reshape(2 * N_LAYERS, 4, 128).transpose(2, 0, 1).reshape(128, 8 * N_LAYERS))
    ident = np.eye(128, dtype=np.float32)

    ekey = (owner * NB + blk) * NPART + part
    order = np.lexsort((ekey,))
    e_sorted = order
    bounds = np.searchsorted(ekey[order], np.arange(CORES * NB * NPART + 1))

    in_maps = []
    for c in range(CORES):
        Sf = np.zeros((NB, 128, CPADMAX * 128), np.float32)
        idxd = np.zeros((128, NB * CPADMAX * 8), np.int16)
        for b in range(NB):
            for p in range(NPART):
                n_chunks = int(cnt[b, p])
                lo_e = bounds[(c * NB + b) * NPART + p]
                hi_e = bounds[(c * NB + b) * NPART + p + 1]
                e = e_sorted[lo_e:hi_e]
                # Dedup src rows within (block, part); S carries multiplicity.
                uniq, inv = np.unique(row_local[e], return_inverse=True)
                n = len(uniq)
                glist = np.zeros(n_chunks * 128, np.int16)  # pad -> row 0
                glist[:n] = uniq.astype(np.int16)
                co = int(off[b, p])
                np.add.at(Sf[b], (inv % 128,
                                  (co + inv // 128) * 128 + slot[e]), 1.0)
                w = glist.reshape(n_chunks * 8, 16).T  # w[p, s] = glist[s*16+p]
                cbase = (b * CPADMAX + co) * 8
                idxd[:, cbase:cbase + n_chunks * 8] = np.tile(w, (8, 1))
        xT_own = np.zeros((D, PADS), np.float32)
        xT_own[:, :SHARD] = x[c * SHARD:(c + 1) * SHARD].T
        in_maps.append({
            "xg": xg_pad,
            "xT": xT_own,
            "Wd": Wd,
            "bT": bT,
            "ident": ident,
            "Sd": Sf.astype(FP8),
            "idxd": idxd,
        })
    meta = {
        "cnt": cnt, "off": off, "CPAD": CPAD, "CPADMAX": CPADMAX,
        "part_base": part_base,
    }
    return in_maps, meta


def build_program(meta):
    import concourse.bacc as bacc
    import concourse.bass as bass
    import concourse.mybir as mybir
    import concourse.tile as tile

    dt = mybir.dt
    f32, bf16, i16, fp8 = dt.float32, dt.bfloat16, dt.int16, dt.float8e4
    AF = mybir.ActivationFunctionType
    DR = mybir.MatmulPerfMode.DoubleRow

    cnt, off, CPAD, CPADMAX = meta["cnt"], meta["off"], meta["CPAD"], meta["CPADMAX"]
    part_base = meta["part_base"]
    NPART = len(PARTS)

    nc = bacc.Bacc("TRN2", target_bir_lowering=False, debug=False,
                   enable_asserts=False, num_devices=CORES, num_swdge_queues=4,
                   dynamic_dma_scratch_size=32768)

    xg = nc.dram_tensor("xg", [PADN, D], fp8, kind="ExternalInput")
    xT = nc.dram_tensor("xT", [D, PADS], f32, kind="ExternalInput")
    Wd = nc.dram_tensor("Wd", [2 * N_LAYERS, D, D], bf16, kind="ExternalInput")
    bTd = nc.dram_tensor("bT", [128, 8 * N_LAYERS], f32, kind="ExternalInput")
    identd = nc.dram_tensor("ident", [128, 128], f32, kind="ExternalInput")
    Sd = nc.dram_tensor("Sd", [NB, 128, CPADMAX * 128], fp8, kind="ExternalInput")
    idxd = nc.dram_tensor("idxd", [128, NB * CPADMAX * 8], i16, kind="ExternalInput")
    outTd = nc.dram_tensor("outT", [D, PADS], f32, kind="ExternalOutput")

    with tile.TileContext(nc) as tc, ExitStack() as ctx:
        p_const = ctx.enter_context(tc.tile_pool(name="const", bufs=1))
        p_big = ctx.enter_context(tc.tile_pool(name="big", bufs=1))
        p_g = ctx.enter_context(tc.tile_pool(name="gth", bufs=6))
        p_aggn = ctx.enter_context(tc.tile_pool(name="aggn", bufs=3))
        p_w = ctx.enter_context(tc.tile_pool(name="wts", bufs=2))
        p_hbf = ctx.enter_context(tc.tile_pool(name="hbf", bufs=3))
        p_aggps = ctx.enter_context(tc.tile_pool(name="aggps", bufs=2, space="PSUM"))
        p_tps = ctx.enter_context(tc.tile_pool(name="tps", bufs=4, space="PSUM"))
        p_mlpps = ctx.enter_context(tc.tile_pool(name="mlpps", bufs=2, space="PSUM"))
        p_dram = ctx.enter_context(tc.tile_pool(name="dram", bufs=1, space="DRAM"))

        # Init loads ordered by first use: indices/S (block-0 gather+agg), hT,
        # ident (transposes), bias table; weights are loaded per layer later.
        idxs = p_const.tile([128, NB * CPADMAX * 8], i16)
        nc.sync.dma_start(idxs[:], idxd.ap())
        S_all = p_const.tile([128, NB, CPADMAX, 128], fp8)
        for b in range(NB):
            C = int(CPAD[b])
            nc.sync.dma_start(S_all[:, b, :C, :], Sd.ap()[b, :, :C * 128])
        ident = p_const.tile([128, 128], f32)
        nc.sync.dma_start(ident[:], identd.ap())

        hT = p_big.tile([128, 4, PADS], f32)     # resident h^T (fp32)
        ZT = p_big.tile([128, 4, PADS], bf16)    # (h + agg)^T, bf16 for MLP
        Y1T = p_big.tile([128, 4, PADS], bf16)   # hidden activation^T
        for kc in range(4):
            nc.sync.dma_start(hT[:, kc, :], xT.ap()[kc * 128:(kc + 1) * 128, :])
        bt = p_const.tile([128, 8 * N_LAYERS], f32)
        nc.sync.dma_start(bt[:], bTd.ap())

        wa_in = p_dram.tile([128, D], i16, name="wa_in")
        wa_out = p_dram.tile([128 * CORES, D], i16, addr_space="Shared", name="wa_out")
        nc.sync.dma_start(wa_in[:, :], xT.ap()[0:128, 0:D].bitcast(i16)[:, 0:D])
        nc.gpsimd.collective_compute(
            "AllGather", mybir.AluOpType.bypass,
            replica_groups=[list(range(CORES))],
            ins=[wa_in.opt()], outs=[wa_out.opt()])

        h_shard = [p_dram.tile([PADS, D], fp8, name=f"hsh{l}") for l in range(2)]
        # One Shared tile per AG part (single-writer rule for Shared DRAM).
        ag_part = [[p_dram.tile([CORES * (hi - lo), D], fp8, addr_space="Shared",
                                name=f"ag{l}_{p}")
                    for p, (lo, hi) in enumerate(PARTS)] for l in range(2)]

        # ---- prepared gathers ------------------------------------------------
        # Unit = (layer, part, block, chunk offset in block, n chunks),
        # block-major: each block's part sub-gathers are consecutive.
        # With sem= the prepared-gather DMA completion is USER-synced.
        # 8 completion "lanes" (sems), one outstanding gather per lane: each
        # prep waits for the previous same-lane gather to complete, so a
        # consumer's wait on 16*k is unambiguous. Sems persist across NEFF
        # executions -> clear at program start.
        NLANE = 8
        dma_sems = [nc.alloc_semaphore(f"gsem{i}") for i in range(NLANE)]
        for i in range(NLANE):
            nc.gpsimd.sem_clear(dma_sems[i])
        lane_uses = [0] * NLANE   # completed-prep count per lane (emission time)
        unit_wait = {}            # unit seq -> (lane, 16*count) consumer wait
        units = []
        block_units = {}  # (l, b) -> [unit indices]
        for l in range(N_LAYERS):
            for b in range(NB):
                block_units[(l, b)] = []
                for p in range(NPART):
                    c0, n = int(off[b, p]), int(cnt[b, p])
                    nsub = -(-n // MAXCH)
                    while n > 0:
                        take = -(-n // nsub)
                        block_units[(l, b)].append(len(units))
                        units.append((l, p, b, c0, take))
                        c0 += take
                        n -= take
                        nsub -= 1
        g_tiles = [None] * NB  # per-block gather tile of the current layer

        def emit_prep(seq):
            l, p, b, c0, nch = units[seq]
            q = seq % 4
            lane = seq % NLANE
            if seq == block_units[(l, b)][0]:
                g_tiles[b] = p_g.tile([128, CPADMAX, D], fp8, tag="g", name="g")
            g = g_tiles[b]
            if l == 0:
                gsrc = xg.ap()[part_base[p]:part_base[p] +
                               CORES * (PARTS[p][1] - PARTS[p][0]), :]
            else:
                gsrc = ag_part[l - 1][p][:, :]
            cbase = (b * CPADMAX + c0) * 8
            inst = nc.gpsimd.dma_gather(
                out_ap=g[:, c0:c0 + nch, :],
                in_ap=gsrc,
                idxs_ap=idxs[:, cbase:cbase + nch * 8],
                num_idxs=nch * 128,
                num_idxs_reg=nch * 128,
                elem_size=D,
                single_packet=False,
                prepare_only=True,
                sem=dma_sems[lane],
                queue_num=q,
            )
            if lane_uses[lane] > 0:
                # One outstanding gather per lane: gate on the previous
                # same-lane completion (its trigger precedes this prep).
                inst._wait_ge(dma_sems[lane], 16 * lane_uses[lane])
            lane_uses[lane] += 1
            unit_wait[seq] = (lane, 16 * lane_uses[lane])

        def emit_trigger(seq):
            nc.gpsimd.trigger_dma(count=None, queue_num=seq % 4)
            # Same-layer lookahead: prep the unit LOOKAHEAD ahead.
            nxt = seq + LOOKAHEAD
            if nxt < len(units) and units[nxt][0] == units[seq][0]:
                emit_prep(nxt)

        for seq in range(LOOKAHEAD):
            emit_prep(seq)

        for l in range(N_LAYERS):
            # ---- aggregation: agg[node, feat] per 128-node dst block ----
            for b in range(NB):
                C = int(CPAD[b])
                for seq in block_units[(l, b)]:
                    emit_trigger(seq)
                # Gather-completion waits (user-synced prepared DMAs).
                need = {}
                for seq in block_units[(l, b)]:
                    lane, val = unit_wait[seq]
                    need[lane] = max(need.get(lane, 0), val)
                for lane, val in sorted(need.items()):
                    nc.tensor.wait_ge(dma_sems[lane], val)
                g = g_tiles[b]
                ps = p_aggps.tile([128, D], f32, name="ps")
                nmm = C // 2 + (C % 2)
                i = 0
                cc = 0
                while cc < C:
                    n = 2 if C - cc >= 2 else 1
                    st, sp = (i == 0), (i == nmm - 1)
                    if n == 2:
                        nc.tensor.matmul(ps[:], lhsT=S_all[:, b, cc:cc + 2, :],
                                         rhs=g[:, cc:cc + 2, :], start=st, stop=sp,
                                         perf_mode=DR)
                    else:
                        nc.tensor.matmul(ps[:], lhsT=S_all[:, b, cc, :],
                                         rhs=g[:, cc, :], start=st, stop=sp)
                    cc += n
                    i += 1
                aggN = p_aggn.tile([128, D], f32, name="aggN")
                nc.scalar.copy(aggN[:], ps[:])
                for fc in range(4):
                    pt = p_tps.tile([128, 128], f32, tag="t", name="pt")
                    nc.tensor.transpose(pt[:], aggN[:, fc * 128:(fc + 1) * 128], ident[:])
                    nc.vector.tensor_add(ZT[:, fc, b * 128:(b + 1) * 128], pt[:],
                                         hT[:, fc, b * 128:(b + 1) * 128])

            # ---- MLP (feature-major, bf16) + chunked AllGather ----
            W0t = p_w.tile([128, 4, D], bf16, tag="w", name="W0t")
            W1t = p_w.tile([128, 4, D], bf16, tag="w", name="W1t")
            for kc in range(4):
                nc.sync.dma_start(W0t[:, kc, :], Wd.ap()[2 * l, kc * 128:(kc + 1) * 128, :])
                nc.sync.dma_start(W1t[:, kc, :], Wd.ap()[2 * l + 1, kc * 128:(kc + 1) * 128, :])
            for ck, (nofs, nw) in enumerate(NCHUNK):
                for j in range(2):
                    rhs_big = ZT if j == 0 else Y1T
                    Wt = W0t if j == 0 else W1t
                    for mc in range(4):
                        ps2 = p_mlpps.tile([128, D], f32, tag="mlp", name="ps2")
                        for kc in range(4):
                            nc.tensor.matmul(
                                ps2[:, :nw],
                                lhsT=Wt[:, kc, mc * 128:(mc + 1) * 128],
                                rhs=rhs_big[:, kc, nofs:nofs + nw],
                                start=(kc == 0), stop=(kc == 3))
                        col = (2 * l + j) * 4 + mc
                        bias = bt[:, col:col + 1]
                        if j == 0:
                            nc.scalar.activation(Y1T[:, mc, nofs:nofs + nw],
                                                 ps2[:, :nw], AF.Relu, bias=bias)
                        elif l < N_LAYERS - 1:
                            nc.scalar.activation(hT[:, mc, nofs:nofs + nw],
                                                 ps2[:, :nw], AF.Relu, bias=bias)
                        else:
                            ot = p_hbf.tile([128, 512], f32, tag="ot", name="ot")
                            nc.scalar.activation(ot[:, :nw], ps2[:, :nw],
                                                 AF.Identity, bias=bias)
                            nc.sync.dma_start(
                                outTd.ap()[mc * 128:(mc + 1) * 128, nofs:nofs + nw],
                                ot[:, :nw])

                if l < N_LAYERS - 1:
                    # h^T -> node-major fp8 rows for this chunk, then AG part.
                    for b in range(nofs // 128, (nofs + nw) // 128):
                        hb = p_hbf.tile([128, D], fp8, tag="hbf", name="hb")
                        for fc in range(4):
                            pt2 = p_tps.tile([128, 128], f32, tag="t", name="pt2")
                            nc.tensor.transpose(pt2[:], hT[:, fc, b * 128:(b + 1) * 128],
                                                ident[:])
                            nc.scalar.copy(hb[:, fc * 128:(fc + 1) * 128], pt2[:])
                        nc.sync.dma_start(h_shard[l][b * 128:(b + 1) * 128, :], hb[:])
                    for p, (lo, hi) in enumerate(PARTS):
                        if hi == nofs + nw:  # part fully stored at this chunk
                            nc.gpsimd.collective_compute(
                                "AllGather",
                                mybir.AluOpType.bypass,
                                replica_groups=[list(range(CORES))],
                                ins=[h_shard[l][lo:hi, :].opt()],
                                outs=[ag_part[l][p].opt()],
                            )

            if l < N_LAYERS - 1:
                # First preps of the next layer (after the AG writes so the
                # deferred RAW dep lands on the triggers).
                u0 = block_units[(l + 1, 0)][0]
                for seq in range(u0, u0 + LOOKAHEAD):
                    emit_prep(seq)

    nc.compile()
    return nc


def kernel(**inputs):
    global LAST_RESULTS
    from concourse import bass_utils

    in_maps, meta = _prep_host(
        inputs["x"], inputs["edge_index"], inputs["Ws"], inputs["bs"])
    nc = build_program(meta)
    res = bass_utils.run_bass_kernel_spmd(
        nc, in_maps, core_ids=list(range(CORES)),
        trace=bool(int(os.environ.get("GIN_TRACE", "0"))),
        tmpdir=os.environ.get("GIN_TMPDIR"),
    )
    LAST_RESULTS = res
    out = np.empty((N_NODES, D), np.float32)
    for c in range(CORES):
        out[c * SHARD:(c + 1) * SHARD] = res.results[c]["outT"][:, :SHARD].T
    return out


# revision 27
# speedup vs baseline: 1.1741x; 1.1741x over previous
"""GIN (3-layer) Trainium2 Bass kernel, 8-core SPMD — v2.

Sharding: nodes (and their incident edges, by dst) are partitioned across the
8 cores; segment_sum is computed locally per dst shard; node features are
exchanged between layers with chunked AllGathers; MLP weights are replicated.

v2 changes over the baseline:
  - gathered node features travel in fp8 (e4m3): halves the dominant HBM
    gather traffic and the inter-core AllGather bytes (rel-err budget checked
    against the reference: ~6e-3 vs 2e-2 tolerance).
  - one-hot segment-sum matmuls run in fp8 with DoubleRow perf mode.
  - selector matrices S are fp8 and stay resident in SBUF across all layers.
  - gathers use prepare_only descriptors + trigger_dma so descriptor
    generation runs ahead of the data dependency (which moves to the trigger).
  - the h AllGather is split into 3 parts aligned with the MLP node chunks;
    each part is a separate Shared tile, and the next layer's gathers are
    split per part so part-p gathers fire as soon as AG part p lands.
"""

import os
import sys
from contextlib import ExitStack

import numpy as np

for _p in ("/opt/trn_rl_repo", "/root/.axon_site/_ro/trn_rl_repo"):
    if os.path.isdir(_p) and _p not in sys.path:
        sys.path.append(_p)

import ml_dtypes

N_NODES = 10000
N_EDGES = 160000
D = 512
N_LAYERS = 3
CORES = 8
SHARD = N_NODES // CORES          # 1250 nodes per core
PADS = 1280                       # padded shard (multiple of 128)
PADN = CORES * PADS               # padded full node count (10240)
NB = PADS // 128                  # dst blocks per core (10)

# AllGather parts: (local row lo, local row hi) — fired at MLP chunk bounds.
PARTS = [(0, 512), (512, 1280)]
NCHUNK = [(0, 512), (512, 512), (1024, 256)]  # node-dim tiles for MLP / AG
MAXCH = 8                         # max chunks per gather (SWDGE ring: 1024 descs)
LOOKAHEAD = 2                     # prep emission distance (units) behind triggers

BF16 = ml_dtypes.bfloat16
FP8 = ml_dtypes.float8_e4m3

# Results of the last kernel() call (BassKernelResults) for the test harness.
LAST_RESULTS = None


def _prep_host(x, edge_index, Ws, bs):
    """Per-core input maps + per-(block, part) chunk counts (uniform across
    cores). Gather rows are part-local: row = rank * part_rows + local offset.
    """
    x = np.asarray(x, np.float32)
    src = np.asarray(edge_index[0], np.int64)
    dst = np.asarray(edge_index[1], np.int64)
    Ws = np.asarray(Ws, np.float32)
    bs = np.asarray(bs, np.float32)

    r_src = src // SHARD
    i_src = src % SHARD
    pedges = np.array([lo for lo, _ in PARTS[1:]], np.int64)
    part = np.searchsorted(pedges, i_src, side="right")
    pw = np.array([hi - lo for lo, hi in PARTS], np.int64)
    plo = np.array([lo for lo, _ in PARTS], np.int64)
    row_local = r_src * pw[part] + (i_src - plo[part])
    ROWMOD = 8192  # > max row_local; keeps dedup keys collision-free

    owner = dst // SHARD
    li = dst % SHARD
    blk = li // 128
    slot = li - blk * 128

    # Per (core, block, part) unique-src counts -> chunk counts (max over
    # cores so the SPMD program is uniform).
    NPART = len(PARTS)
    key = ((owner * NB + blk) * NPART + part) * ROWMOD + row_local
    ucnt = np.zeros(CORES * NB * NPART, np.int64)
    kb = np.unique(key) // ROWMOD
    np.add.at(ucnt, kb, 1)
    ucnt = ucnt.reshape(CORES, NB, NPART)
    cnt = np.maximum(1, -(-ucnt.max(axis=0) // 128))   # [NB, NPART] chunks
    off = np.zeros((NB, NPART + 1), np.int64)
    off[:, 1:] = np.cumsum(cnt, axis=1)
    CPAD = off[:, -1]                                  # padded chunks per block
    CPADMAX = int(CPAD.max())

    # Full part-layout x in fp8 (gather source for layer 0), shared by cores.
    xg_pad = np.zeros((PADN, D), FP8)
    base = 0
    part_base = []
    for (lo, hi) in PARTS:
        part_base.append(base)
        for r in range(CORES):
            hi_c = min(hi, SHARD)
            xg_pad[base + r * (hi - lo): base + r * (hi - lo) + (hi_c - lo)] = \
                x[r * SHARD + lo: r * SHARD + hi_c].astype(FP8)
        base += CORES * (hi - lo)

    Wd = np.ascontiguousarray(Ws.reshape(2 * N_LAYERS, D, D).astype(BF16))
    bT = np.ascontiguousarray(
        bs.reshape(2 * N_LAYERS, 4, 128).transpose(2, 0, 1).reshape(128, 8 * N_LAYERS))
    ident = np.eye(128, dtype=np.float32)

    ekey = (owner * NB + blk) * NPART + part
    order = np.lexsort((ekey,))
    e_sorted = order
    bounds = np.searchsorted(ekey[order], np.arange(CORES * NB * NPART + 1))

    in_maps = []
    for c in range(CORES):
        Sf = np.zeros((NB, 128, CPADMAX * 128), np.float32)
        idxd = np.zeros((128, NB * CPADMAX * 8), np.int16)
        for b in range(NB):
            for p in range(NPART):
                n_chunks = int(cnt[b, p])
                lo_e = bounds[(c * NB + b) * NPART + p]
                hi_e = bounds[(c * NB + b) * NPART + p + 1]
                e = e_sorted[lo_e:hi_e]
                # Dedup src rows within (block, part); S carries multiplicity.
                uniq, inv = np.unique(row_local[e], return_inverse=True)
                n = len(uniq)
                glist = np.zeros(n_chunks * 128, np.int16)  # pad -> row 0
                glist[:n] = uniq.astype(np.int16)
                co = int(off[b, p])
                np.add.at(Sf[b], (inv % 128,
                                  (co + inv // 128) * 128 + slot[e]), 1.0)
                w = glist.reshape(n_chunks * 8, 16).T  # w[p, s] = glist[s*16+p]
                cbase = (b * CPADMAX + co) * 8
                idxd[:, cbase:cbase + n_chunks * 8] = np.tile(w, (8, 1))
        xT_own = np.zeros((D, PADS), np.float32)
        xT_own[:, :SHARD] = x[c * SHARD:(c + 1) * SHARD].T
        in_maps.append({
            "xg": xg_pad,
            "xT": xT_own,
            "Wd": Wd,
            "bT": bT,
            "ident": ident,
            "Sd": Sf.astype(FP8),
            "idxd": idxd,
        })
    meta = {
        "cnt": cnt, "off": off, "CPAD": CPAD, "CPADMAX": CPADMAX,
        "part_base": part_base,
    }
    return in_maps, meta


def build_program(meta):
    import concourse.bacc as bacc
    import concourse.bass as bass
    import concourse.mybir as mybir
    import concourse.tile as tile

    dt = mybir.dt
    f32, bf16, i16, fp8 = dt.float32, dt.bfloat16, dt.int16, dt.float8e4
    AF = mybir.ActivationFunctionType
    DR = mybir.MatmulPerfMode.DoubleRow

    cnt, off, CPAD, CPADMAX = meta["cnt"], meta["off"], meta["CPAD"], meta["CPADMAX"]
    part_base = meta["part_base"]
    NPART = len(PARTS)

    nc = bacc.Bacc("TRN2", target_bir_lowering=False, debug=False,
                   enable_asserts=False, num_devices=CORES, num_swdge_queues=4,
                   dynamic_dma_scratch_size=32768)

    xg = nc.dram_tensor("xg", [PADN, D], fp8, kind="ExternalInput")
    xT = nc.dram_tensor("xT", [D, PADS], f32, kind="ExternalInput")
    Wd = nc.dram_tensor("Wd", [2 * N_LAYERS, D, D], bf16, kind="ExternalInput")
    bTd = nc.dram_tensor("bT", [128, 8 * N_LAYERS], f32, kind="ExternalInput")
    identd = nc.dram_tensor("ident", [128, 128], f32, kind="ExternalInput")
    Sd = nc.dram_tensor("Sd", [NB, 128, CPADMAX * 128], fp8, kind="ExternalInput")
    idxd = nc.dram_tensor("idxd", [128, NB * CPADMAX * 8], i16, kind="ExternalInput")
    outTd = nc.dram_tensor("outT", [D, PADS], f32, kind="ExternalOutput")

    with tile.TileContext(nc) as tc, ExitStack() as ctx:
        p_const = ctx.enter_context(tc.tile_pool(name="const", bufs=1))
        p_big = ctx.enter_context(tc.tile_pool(name="big", bufs=1))
        p_g = ctx.enter_context(tc.tile_pool(name="gth", bufs=6))
        p_aggn = ctx.enter_context(tc.tile_pool(name="aggn", bufs=3))
        p_w = ctx.enter_context(tc.tile_pool(name="wts", bufs=2))
        p_hbf = ctx.enter_context(tc.tile_pool(name="hbf", bufs=3))
        p_aggps = ctx.enter_context(tc.tile_pool(name="aggps", bufs=2, space="PSUM"))
        p_tps = ctx.enter_context(tc.tile_pool(name="tps", bufs=4, space="PSUM"))
        p_mlpps = ctx.enter_context(tc.tile_pool(name="mlpps", bufs=2, space="PSUM"))
        p_dram = ctx.enter_context(tc.tile_pool(name="dram", bufs=1, space="DRAM"))

        # Init loads ordered by first use: indices/S (block-0 gather+agg), hT,
        # ident (transposes), bias table; weights are loaded per layer later.
        idxs = p_const.tile([128, NB * CPADMAX * 8], i16)
        nc.sync.dma_start(idxs[:], idxd.ap())
        S_all = p_const.tile([128, NB, CPADMAX, 128], fp8)
        for b in range(NB):
            C = int(CPAD[b])
            nc.sync.dma_start(S_all[:, b, :C, :], Sd.ap()[b, :, :C * 128])
        ident = p_const.tile([128, 128], f32)
        nc.sync.dma_start(ident[:], identd.ap())

        hT = p_big.tile([128, 4, PADS], f32)     # resident h^T (fp32)
        ZT = p_big.tile([128, 4, PADS], bf16)    # (h + agg)^T, bf16 for MLP
        Y1T = p_big.tile([128, 4, PADS], bf16)   # hidden activation^T
        for kc in range(4):
            nc.sync.dma_start(hT[:, kc, :], xT.ap()[kc * 128:(kc + 1) * 128, :])
        bt = p_const.tile([128, 8 * N_LAYERS], f32)
        nc.sync.dma_start(bt[:], bTd.ap())

        wa_in = p_dram.tile([128, D], i16, name="wa_in")
        wa_out = p_dram.tile([128 * CORES, D], i16, addr_space="Shared", name="wa_out")
        nc.sync.dma_start(wa_in[:, :], xT.ap()[0:128, 0:D].bitcast(i16)[:, 0:D])
        nc.gpsimd.collective_compute(
            "AllGather", mybir.AluOpType.bypass,
            replica_groups=[list(range(CORES))],
            ins=[wa_in.opt()], outs=[wa_out.opt()])

        h_shard = [p_dram.tile([PADS, D], fp8, name=f"hsh{l}") for l in range(2)]
        # One Shared tile per AG part (single-writer rule for Shared DRAM).
        ag_part = [[p_dram.tile([CORES * (hi - lo), D], fp8, addr_space="Shared",
                                name=f"ag{l}_{p}")
                    for p, (lo, hi) in enumerate(PARTS)] for l in range(2)]

        # ---- gathers: one unit per (layer, part, block) sub-range -----------
        # Normal (Tile-managed) gathers: desc-gen is cheap; DMASW lanes give
        # the consumer sync and the SWDGE ring paces in-flight units.
        units = []
        block_units = {}  # (l, b) -> [unit indices]
        for l in range(N_LAYERS):
            for b in range(NB):
                block_units[(l, b)] = []
                for p in range(NPART):
                    c0, n = int(off[b, p]), int(cnt[b, p])
                    nsub = -(-n // MAXCH)
                    while n > 0:
                        take = -(-n // nsub)
                        block_units[(l, b)].append(len(units))
                        units.append((l, p, b, c0, take))
                        c0 += take
                        n -= take
                        nsub -= 1
        g_tiles = [None] * NB  # per-block gather tile of the current layer

        def emit_gather(seq):
            l, p, b, c0, nch = units[seq]
            if seq == block_units[(l, b)][0]:
                g_tiles[b] = p_g.tile([128, CPADMAX, D], fp8, tag="g", name="g")
            g = g_tiles[b]
            if l == 0:
                gsrc = xg.ap()[part_base[p]:part_base[p] +
                               CORES * (PARTS[p][1] - PARTS[p][0]), :]
            else:
                gsrc = ag_part[l - 1][p][:, :]
            cbase = (b * CPADMAX + c0) * 8
            nc.gpsimd.dma_gather(
                out_ap=g[:, c0:c0 + nch, :],
                in_ap=gsrc,
                idxs_ap=idxs[:, cbase:cbase + nch * 8],
                num_idxs=nch * 128,
                num_idxs_reg=nch * 128,
                elem_size=D,
                single_packet=False,
                queue_num=seq % 4,
            )

        for l in range(N_LAYERS):
            # ---- aggregation: agg[node, feat] per 128-node dst block ----
            for b in range(NB):
                C = int(CPAD[b])
                for seq in block_units[(l, b)]:
                    emit_gather(seq)
                g = g_tiles[b]
                ps = p_aggps.tile([128, D], f32, name="ps")
                nmm = C // 2 + (C % 2)
                i = 0
                cc = 0
                while cc < C:
                    n = 2 if C - cc >= 2 else 1
                    st, sp = (i == 0), (i == nmm - 1)
                    if n == 2:
                        nc.tensor.matmul(ps[:], lhsT=S_all[:, b, cc:cc + 2, :],
                                         rhs=g[:, cc:cc + 2, :], start=st, stop=sp,
                                         perf_mode=DR)
                    else:
                        nc.tensor.matmul(ps[:], lhsT=S_all[:, b, cc, :],
                                         rhs=g[:, cc, :], start=st, stop=sp)
                    cc += n
                    i += 1
                aggN = p_aggn.tile([128, D], f32, name="aggN")
                nc.scalar.copy(aggN[:], ps[:])
                for fc in range(4):
                    pt = p_tps.tile([128, 128], f32, tag="t", name="pt")
                    nc.tensor.transpose(pt[:], aggN[:, fc * 128:(fc + 1) * 128], ident[:])
                    nc.vector.tensor_add(ZT[:, fc, b * 128:(b + 1) * 128], pt[:],
                                         hT[:, fc, b * 128:(b + 1) * 128])

            # ---- MLP (feature-major, bf16) + chunked AllGather ----
            W0t = p_w.tile([128, 4, D], bf16, tag="w", name="W0t")
            W1t = p_w.tile([128, 4, D], bf16, tag="w", name="W1t")
            for kc in range(4):
                nc.sync.dma_start(W0t[:, kc, :], Wd.ap()[2 * l, kc * 128:(kc + 1) * 128, :])
                nc.sync.dma_start(W1t[:, kc, :], Wd.ap()[2 * l + 1, kc * 128:(kc + 1) * 128, :])
            for ck, (nofs, nw) in enumerate(NCHUNK):
                for j in range(2):
                    rhs_big = ZT if j == 0 else Y1T
                    Wt = W0t if j == 0 else W1t
                    for mc in range(4):
                        ps2 = p_mlpps.tile([128, D], f32, tag="mlp", name="ps2")
                        for kc in range(4):
                            nc.tensor.matmul(
                                ps2[:, :nw],
                                lhsT=Wt[:, kc, mc * 128:(mc + 1) * 128],
                                rhs=rhs_big[:, kc, nofs:nofs + nw],
                                start=(kc == 0), stop=(kc == 3))
                        col = (2 * l + j) * 4 + mc
                        bias = bt[:, col:col + 1]
                        if j == 0:
                            nc.scalar.activation(Y1T[:, mc, nofs:nofs + nw],
                                                 ps2[:, :nw], AF.Relu, bias=bias)
                        elif l < N_LAYERS - 1:
                            nc.scalar.activation(hT[:, mc, nofs:nofs + nw],
                                                 ps2[:, :nw], AF.Relu, bias=bias)
                        else:
                            ot = p_hbf.tile([128, 512], f32, tag="ot", name="ot")
                            nc.scalar.activation(ot[:, :nw], ps2[:, :nw],
                                                 AF.Identity, bias=bias)
                            nc.sync.dma_start(
                                outTd.ap()[mc * 128:(mc + 1) * 128, nofs:nofs + nw],
                                ot[:, :nw])

                if l < N_LAYERS - 1:
                    # h^T -> node-major fp8 rows for this chunk, then AG part.
                    for b in range(nofs // 128, (nofs + nw) // 128):
                        hb = p_hbf.tile([128, D], fp8, tag="hbf", name="hb")
                        for fc in range(4):
                            pt2 = p_tps.tile([128, 128], f32, tag="t", name="pt2")
                            nc.tensor.transpose(pt2[:], hT[:, fc, b * 128:(b + 1) * 128],
                                                ident[:])
                            nc.scalar.copy(hb[:, fc * 128:(fc + 1) * 128], pt2[:])
                        nc.sync.dma_start(h_shard[l][b * 128:(b + 1) * 128, :], hb[:])
                    for p, (lo, hi) in enumerate(PARTS):
                        if hi == nofs + nw:  # part fully stored at this chunk
                            nc.gpsimd.collective_compute(
                                "AllGather",
                                mybir.AluOpType.bypass,
                                replica_groups=[list(range(CORES))],
                                ins=[h_shard[l][lo:hi, :].opt()],
                                outs=[ag_part[l][p].opt()],
                            )



    nc.compile()
    return nc


def kernel(**inputs):
    global LAST_RESULTS
    from concourse import bass_utils

    in_maps, meta = _prep_host(
        inputs["x"], inputs["edge_index"], inputs["Ws"], inputs["bs"])
    nc = build_program(meta)
    res = bass_utils.run_bass_kernel_spmd(
        nc, in_maps, core_ids=list(range(CORES)),
        trace=bool(int(os.environ.get("GIN_TRACE", "0"))),
        tmpdir=os.environ.get("GIN_TMPDIR"),
    )
    LAST_RESULTS = res
    out = np.empty((N_NODES, D), np.float32)
    for c in range(CORES):
        out[c * SHARD:(c + 1) * SHARD] = res.results[c]["outT"][:, :SHARD].T
    return out
